# revision 1
# baseline (speedup 1.0000x reference)
"""Trainium2 Bass kernel for CapLayer2 (1x1-conv capsule layer with dynamic routing).

Sharding: data-parallel over batch — 8 batches per core on 8 NeuronCores.

Per-core design (2 waves x 4 batches):
  - The 1x1 conv produces BOTH pred layouts on TensorE in float32r:
      predT [i-part, o]  (for the s matmuls, contraction over i=1024)
      pred  [o-part, i]  (for the delta matmuls, contraction over o=320)
    The conv bias is folded into the evictions: a DVE tensor-add against a
    partition-broadcast bias tile for predT, and the per-partition bias
    operand of the ScalarE activation for pred.
  - Routing state b/c lives in [i-part, (batch, itile, j)] layout so the
    softmax over j (J=10) is a free-dim grouped reduction; softmax runs
    per batch so each batch's s matmuls start as soon as its own delta
    transposes land (batch-level pipelining).
  - s/delta matmuls use per-batch [10, N] PSUM tiles at base partition 0
    (column tiling is illegal for 4-byte dtypes), so squash norms are
    natural per-partition accumulators (Square with accum_out).
  - sqrt is computed as exp(0.5*ln) and get_activation_tables is pinned to
    natural_log_exp_and_others so the ACT engine never reloads its table.
  - delta [10, 1024] rows are PE-transposed back to [i-part, j] in 128-col
    blocks packed into one PSUM tile, giving a single [128, 80] DVE add
    into b per batch-iteration.
"""

import numpy as np
from contextlib import ExitStack

import concourse.bacc as bacc
import concourse.bass as bass
import concourse.hw_specs as hw_specs

# Force every activation onto the one table that contains all functions this
# kernel uses (Copy/Identity/Exp/Ln/Square) so the ACT engine loads its
# function table exactly once instead of thrashing between sets.
_ONE_TABLE = "natural_log_exp_and_others"
_orig_get_tables = hw_specs.get_activation_tables


def _pinned_tables(arch):
    tabs = _orig_get_tables(arch)
    return {k: (v if k == _ONE_TABLE else set()) for k, v in tabs.items()}


bacc.get_activation_tables = _pinned_tables
import concourse.tile as tile
from concourse import mybir
from concourse.bass_utils import run_bass_kernel_spmd

F32 = mybir.dt.float32
F32R = mybir.dt.float32r
AF = mybir.ActivationFunctionType
OP = mybir.AluOpType

N_CORES = 8
BS = 64
C_IN = 256
J = 10
D = 32
O = J * D          # 320
I = 1024           # 32*32 pixels
ROUTE_NUM = 3
B_PER_CORE = BS // N_CORES   # 8
WAVE = 4
N_WAVES = B_PER_CORE // WAVE
N_IT = I // 128    # 8
N_KT = C_IN // 128 # 2
N_OT = 3           # o tiles: 128, 128, 64


def r(ap):
    return ap.bitcast(F32R)


def strip_gather(t, kw=128):
    """[kw, 128] tile -> [kw, WAVE, J] AP selecting cols 32*b+j."""
    return bass.AP(tensor=t.tensor, offset=t.offset, ap=[list(t.ap[0]), [32, WAVE], [1, J]])[:kw]


def build_kernel(stage=5):
    nc = bacc.Bacc("TRN2", target_bir_lowering=False, debug=False, num_devices=1)

    x_d = nc.dram_tensor("x", [B_PER_CORE, C_IN, I], F32R, kind="ExternalInput")
    wt_d = nc.dram_tensor("wt", [C_IN, O], F32R, kind="ExternalInput")   # W.T
    wb_d = nc.dram_tensor("wb", [1, O], F32R, kind="ExternalInput")
    out_d = nc.dram_tensor("v", [B_PER_CORE, J, D], F32, kind="ExternalOutput")

    ident_np = np.eye(128, dtype=np.float32)
    bm = np.zeros((128, O), dtype=np.float32)
    for b4 in range(WAVE):
        for j in range(J):
            bm[32 * b4 + j, 32 * j:32 * j + 32] = 1.0
    ident_d = nc.inline_tensor(ident_np, name="ident")
    bmask_d = nc.inline_tensor(bm, name="bmask")
    c0_d = nc.inline_tensor(np.full((128, J), 1.0 / J, dtype=np.float32), name="c0")

    with tile.TileContext(nc) as tc:
        with ExitStack() as ctx:
            consts = ctx.enter_context(tc.tile_pool(name="consts", bufs=1))
            xpool = ctx.enter_context(tc.tile_pool(name="xp", bufs=3))
            ppool = ctx.enter_context(tc.tile_pool(name="pp", bufs=WAVE + 2))
            state = ctx.enter_context(tc.tile_pool(name="st", bufs=2))
            ps_conv = ctx.enter_context(tc.tile_pool(name="psc", bufs=2, space="PSUM"))
            ps_st = ctx.enter_context(tc.tile_pool(name="pss", bufs=1, space="PSUM"))
            ps_tp = ctx.enter_context(tc.tile_pool(name="pst", bufs=3, space="PSUM"))
            ps_dp = ctx.enter_context(tc.tile_pool(name="psd", bufs=2, space="PSUM"))

            # ---- constants ----
            wt_sb = consts.tile([128, N_KT * O], F32R)
            nc.sync.dma_start(
                out=wt_sb.rearrange("p (k o) -> p k o", o=O),
                in_=wt_d.ap().rearrange("(k p) o -> p k o", p=128),
            )
            bias_b128 = consts.tile([128, O], F32)
            wb_bc = bass.AP(
                tensor=wb_d, offset=0, ap=[[0, 128], [1, O]]
            ).bitcast(F32)
            nc.sync.dma_start(out=bias_b128, in_=wb_bc)
            routing_consts = {}

            def load_routing_consts():
                # Emitted after the first batches' x DMAs are enqueued so the
                # small/scatter transfers don't delay the startup-critical x.
                bias_col = consts.tile([128, N_OT], F32)
                for m in range(N_OT):
                    mw = 128 if m < 2 else 64
                    nc.sync.dma_start(
                        out=bias_col[0:mw, m:m + 1],
                        in_=wb_d.ap().bitcast(F32)[0:1, 128 * m:128 * m + mw],
                    )
                ident_sb = consts.tile([128, 16], F32)
                nc.sync.dma_start(out=ident_sb, in_=ident_d.ap()[:, :16])
                bmask_sb = consts.tile([128, O], F32)
                nc.sync.dma_start(out=bmask_sb, in_=bmask_d.ap())
                c0_sb = consts.tile([128, J], F32R)
                nc.sync.dma_start(out=c0_sb, in_=r(c0_d.ap()))
                routing_consts.update(
                    bias_col=bias_col, ident_sb=ident_sb, bmask_sb=bmask_sb,
                    c0_sb=c0_sb,
                )

            for wave in range(N_WAVES):
                # ======== conv: both layouts, 4 batches ========
                predT, pred = [], []
                for b in range(WAVE):
                    gb = wave * WAVE + b
                    x_sb = xpool.tile([128, N_KT * I], F32R, tag="x")
                    for k in range(N_KT):
                        nc.sync.dma_start(
                            out=x_sb[:, k * I:(k + 1) * I],
                            in_=x_d.ap()[gb][k * 128:(k + 1) * 128, :],
                        )
                    if wave == 0 and b == 0:
                        load_routing_consts()
                    bias_col = routing_consts["bias_col"]
                    ident_sb = routing_consts["ident_sb"]
                    bmask_sb = routing_consts["bmask_sb"]
                    c0_sb = routing_consts["c0_sb"]

                    pT = ppool.tile([128, N_IT * O], F32R, tag="predT")
                    for t in range(N_IT):
                        ps = ps_conv.tile([128, 512], F32, tag="cv")
                        for k in range(N_KT):
                            nc.tensor.matmul(
                                ps[:, :O],
                                r(x_sb[:, k * I + t * 128:k * I + t * 128 + 128]),
                                r(wt_sb[:, k * O:(k + 1) * O]),
                                start=(k == 0),
                                stop=(k == N_KT - 1),
                            )
                        # eviction fused with the conv-bias add
                        nc.vector.tensor_tensor(
                            pT[:, t * O:(t + 1) * O], ps[:, :O], bias_b128, OP.add
                        )
                    predT.append(pT)

                    pr = ppool.tile([128, N_OT * I], F32R, tag="pred")
                    for m in range(N_OT):
                        mw = 128 if m < 2 else 64
                        for h in range(2):
                            ps = ps_conv.tile([128, 512], F32, tag="cv")
                            for k in range(N_KT):
                                nc.tensor.matmul(
                                    ps[:mw],
                                    r(wt_sb[:, k * O + m * 128:k * O + m * 128 + mw]),
                                    r(x_sb[:, k * I + h * 512:k * I + h * 512 + 512]),
                                    start=(k == 0),
                                    stop=(k == N_KT - 1),
                                )
                            dst_pr = pr[:mw, m * I + h * 512:m * I + h * 512 + 512]
                            if (m * 2 + h) % 2 == 0:
                                nc.scalar.activation(
                                    dst_pr, ps[:mw], AF.Identity,
                                    bias=bias_col[0:mw, m:m + 1], scale=1.0,
                                )
                            else:
                                nc.vector.tensor_scalar_add(
                                    dst_pr, ps[:mw], bias_col[0:mw, m:m + 1]
                                )
                    pred.append(pr)

                # ======== routing ========
                if stage < 2:
                    for b in range(WAVE):
                        gb = wave * WAVE + b
                        dump = state.tile([128, D], F32, tag="v_cmp")
                        nc.vector.tensor_copy(dump[:J], predT[b][:J, :D])
                        nc.vector.tensor_add(dump[:J], dump[:J], pred[b][:J, :D].bitcast(F32))
                        nc.sync.dma_start(out=out_d.ap()[gb], in_=dump[:J])
                    continue
                b_sb = state.tile([128, WAVE * N_IT * J], F32, tag="b")
                c_sb = state.tile([128, WAVE * N_IT * J], F32R, tag="c")
                for it in range(ROUTE_NUM):
                    last = it == ROUTE_NUM - 1
                    V_sb = state.tile([128, N_OT * WAVE * J], F32R, tag="V")
                    for b in range(WAVE):
                        if it > 0:
                            # per-batch softmax over j (free-dim groups of 10)
                            sl = slice(b * N_IT * J, (b + 1) * N_IT * J)
                            e_sb = state.tile([128, N_IT * J], F32, tag="e")
                            nc.scalar.activation(e_sb, b_sb[:, sl], AF.Exp)
                            den = state.tile([128, N_IT], F32, tag="den")
                            nc.vector.reduce_sum(
                                den,
                                e_sb.rearrange("p (g j) -> p g j", j=J),
                                axis=mybir.AxisListType.X,
                            )
                            rden = state.tile([128, N_IT], F32, tag="rden")
                            nc.vector.reciprocal(rden, den)
                            nc.vector.tensor_tensor(
                                c_sb[:, sl].rearrange("p (g j) -> p g j", j=J),
                                e_sb.rearrange("p (g j) -> p g j", j=J),
                                rden.broadcast_to([128, N_IT, J]),
                                OP.mult,
                            )
                        # ---- s = c . predT  (contraction over i) ----
                        ps_s = ps_st.tile([128, O], F32, tag="s")
                        for t in range(N_IT):
                            lhs = (
                                c0_sb
                                if it == 0
                                else c_sb[:, (b * N_IT + t) * J:(b * N_IT + t + 1) * J]
                            )
                            nc.tensor.matmul(
                                ps_s[:J],
                                r(lhs),
                                r(predT[b][:, t * O:(t + 1) * O]),
                                start=(t == 0),
                                stop=(t == N_IT - 1),
                            )

                        # ---- squash (partitions = j) ----
                        s_m = state.tile([128, O], F32, tag="s_m")
                        nc.vector.tensor_tensor(s_m[:J], ps_s[:J], bmask_sb[:J], OP.mult)
                        sq = state.tile([128, O], F32, tag="sq")
                        ns = state.tile([128, 1], F32, tag="ns")
                        nc.scalar.activation(
                            sq[:J], s_m[:J], AF.Square, accum_out=ns[:J]
                        )
                        # sqrt(ns) = exp(0.5*ln(ns)) — keeps every ACT func
                        # in the natural_log_exp_and_others table (one load,
                        # no per-iteration table thrash)
                        lns = state.tile([128, 1], F32, tag="lns")
                        nc.scalar.activation(lns[:J], ns[:J], AF.Ln)
                        rt = state.tile([128, 1], F32, tag="rt")
                        nc.scalar.activation(rt[:J], lns[:J], AF.Exp, scale=0.5)
                        ns1 = state.tile([128, 1], F32, tag="ns1")
                        nc.vector.tensor_scalar_add(ns1[:J], ns[:J], 1.0)
                        rns1 = state.tile([128, 1], F32, tag="rns1")
                        nc.vector.reciprocal(rns1[:J], ns1[:J])
                        coeff = state.tile([128, 1], F32, tag="coeff")
                        nc.vector.tensor_tensor(coeff[:J], rt[:J], rns1[:J], OP.mult)
                        v_full = state.tile([128, O], F32, tag="v_full")
                        nc.vector.tensor_scalar_mul(v_full[:J], s_m[:J], coeff[:J])

                        if last or stage == 2:
                            v_cmp = state.tile([128, D], F32, tag="v_cmp")
                            nc.vector.reduce_sum(
                                v_cmp[:J],
                                v_full[:J].rearrange("p (j d) -> p d j", j=J),
                                axis=mybir.AxisListType.X,
                            )
                            if last or (stage == 2 and it == 0):
                                gb = wave * WAVE + b
                                nc.sync.dma_start(out=out_d.ap()[gb], in_=v_cmp[:J])
                            continue

                        # ---- V: transpose v into [o-part, (k, b, j)] ----
                        ps_tv = ps_tp.tile([128, N_OT * J], F32, tag="T")
                        nc.vector.memset(ps_tv[64:, 2 * J:3 * J], 0.0)
                        for k in range(N_OT):
                            kw = 128 if k < 2 else 64
                            nc.tensor.transpose(
                                ps_tv[:kw, k * J:(k + 1) * J],
                                v_full[:J, k * 128:k * 128 + kw],
                                ident_sb[:J, :J],
                            )
                        vdst = bass.AP(
                            tensor=V_sb.tensor,
                            offset=V_sb.offset + b * J,
                            ap=[list(V_sb.ap[0]), [WAVE * J, N_OT], [1, J]],
                        )
                        nc.vector.tensor_copy(vdst, ps_tv.rearrange("p (k j) -> p k j", j=J))

                    if last or stage == 2:
                        if stage == 2:
                            break
                        continue

                    if stage == 3:
                        for b in range(WAVE):
                            gb = wave * WAVE + b
                            dmp = state.tile([128, D], F32, tag="v_cmp")
                            nc.vector.tensor_copy(dmp[:J], V_sb[:J, :D].bitcast(F32))
                            nc.sync.dma_start(out=out_d.ap()[gb], in_=dmp[:J])
                        break
                    for b in range(WAVE):
                        # ---- delta = V . pred  (contraction over o) ----
                        delta_sb = state.tile([128, I], F32, tag="delta")
                        for h in range(2):
                            ps_d = ps_dp.tile([128, 512], F32, tag="d")
                            for k in range(N_OT):
                                kw = 128 if k < 2 else 64
                                nc.tensor.matmul(
                                    ps_d[:J],
                                    r(V_sb[:kw, (k * WAVE + b) * J:(k * WAVE + b + 1) * J]),
                                    r(pred[b][:kw, k * I + h * 512:k * I + (h + 1) * 512]),
                                    start=(k == 0),
                                    stop=(k == N_OT - 1),
                                )
                            nc.scalar.copy(delta_sb[:J, h * 512:(h + 1) * 512], ps_d[:J])

                        if stage == 4:
                            gb = wave * WAVE + b
                            dmp2 = state.tile([128, D], F32, tag="v_cmp")
                            nc.vector.tensor_copy(dmp2[:J], delta_sb[:J, :D])
                            nc.sync.dma_start(out=out_d.ap()[gb], in_=dmp2[:J])
                            continue
                        # ---- transpose delta back into [i-part, j] ----
                        ps_t = ps_tp.tile([128, N_IT * J], F32, tag="T")
                        for t in range(N_IT):
                            nc.tensor.transpose(
                                ps_t[:, t * J:(t + 1) * J],
                                delta_sb[:J, t * 128:(t + 1) * 128],
                                ident_sb[:J, :J],
                            )
                        dst = b_sb[:, b * N_IT * J:(b + 1) * N_IT * J]
                        if it == 0:
                            nc.vector.tensor_copy(dst, ps_t)
                        else:
                            nc.vector.tensor_tensor(dst, ps_t, dst, OP.add)
                    if stage == 4:
                        break

    nc.compile()
    return nc


_NC_CACHE = None
LAST_RESULT = None


def kernel(x: np.ndarray, W: np.ndarray, W_b: np.ndarray) -> np.ndarray:
    global _NC_CACHE
    if _NC_CACHE is None:
        _NC_CACHE = build_kernel()
    nc = _NC_CACHE

    x = np.ascontiguousarray(x.reshape(BS, C_IN, I), dtype=np.float32)
    wt = np.ascontiguousarray(W.T, dtype=np.float32)
    wb = np.ascontiguousarray(W_b.reshape(1, O), dtype=np.float32)

    in_maps = [
        {
            "x": np.ascontiguousarray(x[c * B_PER_CORE:(c + 1) * B_PER_CORE]),
            "wt": wt,
            "wb": wb,
        }
        for c in range(N_CORES)
    ]
    import os
    trace = bool(int(os.environ.get("KERNEL_TRACE", "0")))
    res = run_bass_kernel_spmd(
        nc, in_maps, core_ids=list(range(N_CORES)), trace=trace
    )
    if trace:
        global LAST_RESULT
        LAST_RESULT = res
    out = np.concatenate([res.results[c]["v"] for c in range(N_CORES)], axis=0)
    return out.astype(np.float32)


if __name__ == "__main__":
    rng = np.random.default_rng(0)
    x = rng.standard_normal((BS, C_IN, 32, 32), dtype=np.float32)
    W = (rng.standard_normal((O, C_IN)) * 0.02).astype(np.float32)
    W_b = (rng.standard_normal((O,)) * 0.02).astype(np.float32)
    v = kernel(x=x, W=W, W_b=W_b)
    print(v.shape, v.dtype, float(np.abs(v).max()))



# revision 31
# speedup vs baseline: 1.6092x; 1.6092x over previous
"""Trainium2 Bass kernel for CapLayer2 (1x1-conv capsule layer with dynamic routing).

Sharding: data-parallel over batch - 8 batches per core on 8 NeuronCores.

Design notes (driven by the instruction cost model):
  - Matmul cost = out-free-size x pe_cycle; weight (stationary) loads are free.
    So every routing contraction puts the big tensor in the STATIONARY operand
    and streams a 10-column moving operand: s^T = predT^T c (out [o,10]),
    delta = predB^T V (out [i,10]), v = V^T dmask (out [10,32]).
  - The conv computes only predT [i-part, o]; the [o-part, i] layout (predB)
    comes from PE transposes of the f16 predT (1.0 cyc/row, psum out f16),
    whose eviction then runs at the DVE 2x half-cycle rate.
  - All on-chip tensors are float16 (f16 matmuls are full rate; fp32r at
    free-size<256 would be 4x slower; f16 keeps 10 mantissa bits).
  - The conv bias is folded into the routing matmuls as rank-1 updates
    (K=1 matmuls), so all PSUM evictions are pure copies and can be spread
    across DVE/ACT/GPSIMD.
  - o-blocks are [0:128), [128:256), [192:320): all 128 partitions wide.
    The V mask zeroes rows of block1 that alias block2, so the block-2
    overlap contributes zero to delta/v and no PSUM row is ever read stale.
  - squash coeff: ns -> [2,40] (duplicated rows via ones2 lhsT), one ACT Ln
    with per-partition bias (0,1) gives ln(ns) and ln(1+ns), one DVE
    scalar_tensor_tensor forms 0.5*ln(ns)-ln(1+ns), one ACT Exp gives
    coeff = sqrt(ns)/(1+ns); a rank-1 matmul broadcasts it to 128 partitions.
"""

import numpy as np
import ml_dtypes

import concourse.bacc as bacc
import concourse.bass as bass
import concourse.hw_specs as hw_specs

# Keep every activation in one function table so no table switches are emitted.
_ONE_TABLE = "natural_log_exp_and_others"
_orig_get_tables = hw_specs.get_activation_tables


def _pinned_tables(arch):
    tabs = _orig_get_tables(arch)
    return {k: (v if k == _ONE_TABLE else set()) for k, v in tabs.items()}


bacc.get_activation_tables = _pinned_tables
import concourse.tile as tile
from concourse import mybir
from concourse.bass_utils import run_bass_kernel_spmd

F32 = mybir.dt.float32
F32R = mybir.dt.float32r
F16 = mybir.dt.float16
AF = mybir.ActivationFunctionType
OP = mybir.AluOpType
AX = mybir.AxisListType

N_CORES = 8
BS = 64
C_IN = 256
J = 10
D = 32
O = J * D          # 320
I = 1024
ROUTE_NUM = 3
B_PER_CORE = BS // N_CORES   # 8
WAVE = 4
N_WAVES = B_PER_CORE // WAVE
N_IT = I // 128    # 8

# o-blocks for the [o-part] layouts: all 128 wide; block1 rows >=64 (o 192..255)
# are masked out of V so block2 [192:320) supplies them instead.
OLOS = [0, 128, 192]
F16T = np.float16


def ap_of(t, offset_cols, dims):
    """Custom free-dim AP over tile t: dims = [[stride, count], ...]."""
    return bass.AP(tensor=t.tensor, offset=t.offset + offset_cols,
                   ap=[list(t.ap[0])] + [list(d) for d in dims])


def build_kernel(dump=False):
    nc = bacc.Bacc("TRN2", target_bir_lowering=False, debug=False, num_devices=1)

    x_d = nc.dram_tensor("x", [B_PER_CORE, C_IN, I], F32R, kind="ExternalInput")
    wt_d = nc.dram_tensor("wt", [C_IN, O], F32R, kind="ExternalInput")   # W.T
    wb_d = nc.dram_tensor("wb", [1, O], F32, kind="ExternalInput")
    out_d = nc.dram_tensor("v", [B_PER_CORE, J, D], F32, kind="ExternalOutput")
    DT = {}
    if dump:
        DT["predT0"] = nc.dram_tensor("predT0", [128, N_IT * O], F16, kind="ExternalOutput")
        DT["predB0"] = nc.dram_tensor("predB0", [128, 3 * I], F16, kind="ExternalOutput")
        for t in range(ROUTE_NUM):
            DT[f"sm{t}"] = nc.dram_tensor(f"sm{t}", [128, 120], F16, kind="ExternalOutput")
            DT[f"ns{t}"] = nc.dram_tensor(f"ns{t}", [1, 80], F32, kind="ExternalOutput")
            DT[f"co{t}"] = nc.dram_tensor(f"co{t}", [1, 40], F16, kind="ExternalOutput")
            DT[f"V{t}"] = nc.dram_tensor(f"V{t}", [128, 120], F16, kind="ExternalOutput")
        for t in range(2):
            DT[f"b{t}"] = nc.dram_tensor(f"b{t}", [128, 320], F32, kind="ExternalOutput")
            DT[f"bv{t}"] = nc.dram_tensor(f"bv{t}", [1, 40], F16, kind="ExternalOutput")
        for t in (1, 2):
            DT[f"c{t}"] = nc.dram_tensor(f"c{t}", [128, 320], F16, kind="ExternalOutput")

    # ---- inline constants (f16) ----
    ident_np = np.eye(128, dtype=F16T)
    c0v = np.float16(0.1)                      # f16(0.1) = 0.0999755859375
    c0_np = np.full((128, J), c0v, dtype=F16T)
    corr = 0.1 / float(c0v)                    # iter-0 softmax correction
    # sigma0 = sum_i c0[i,j] = 1024*f16(0.1); the bmask0 correction then maps
    # it to the exact 102.4 the f32 reference uses.
    sig0_np = np.full((1, 40), np.float32(I) * c0v, dtype=F16T)
    ones10_np = np.ones((1, J), dtype=F16T)
    ones40_np = np.ones((1, 40), dtype=F16T)
    onesrow_np = np.ones((1, 128), dtype=F16T)
    ones1_np = np.ones((128, 1), dtype=F16T)
    dmask_np = np.zeros((128, D), dtype=F16T)
    for p in range(128):
        dmask_np[p, p % D] = 1.0
    bmask_np = np.zeros((128, 30), dtype=np.float32)
    for ob, olo in enumerate(OLOS):
        for p in range(128):
            o = olo + p
            if ob == 1 and o >= 192:
                continue                       # block2 owns o in [192,320)
            bmask_np[p, ob * J + o // D] = 1.0
    ident_d = nc.inline_tensor(ident_np, name="ident")
    c0_d = nc.inline_tensor(c0_np, name="c0")
    sig0_d = nc.inline_tensor(sig0_np, name="sig0")
    ones10_d = nc.inline_tensor(ones10_np, name="ones10")
    ones40_d = nc.inline_tensor(ones40_np, name="ones40")
    onesrow_d = nc.inline_tensor(onesrow_np, name="onesrow")
    ones1_d = nc.inline_tensor(ones1_np, name="ones1")
    dmask_d = nc.inline_tensor(dmask_np, name="dmask")
    bmask_d = nc.inline_tensor(bmask_np.astype(F16T), name="bmask")
    bmask0_d = nc.inline_tensor((bmask_np * corr).astype(F16T), name="bmask0")

    with tile.TileContext(nc) as tc:
        with nc.allow_low_precision(reason="f16 intermediates, 2e-2 tolerance"), \
             tc.tile_pool(name="consts", bufs=1) as consts, \
             tc.tile_pool(name="xp", bufs=3) as xp, \
             tc.tile_pool(name="ptp", bufs=B_PER_CORE) as ptp, \
             tc.tile_pool(name="pbp", bufs=B_PER_CORE) as pbp, \
             tc.tile_pool(name="st", bufs=2) as st, \
             tc.tile_pool(name="pscv", bufs=2, space="PSUM") as pscv, \
             tc.tile_pool(name="pstp", bufs=2, space="PSUM") as pstp, \
             tc.tile_pool(name="psrt", bufs=2, space="PSUM") as psrt:

            # ---------------- constants ----------------
            CK = {}

            def load_consts():
                wt_sb = consts.tile([128, 2 * O], F32R)
                nc.sync.dma_start(
                    out=wt_sb.rearrange("p (k o) -> p k o", o=O),
                    in_=wt_d.ap().rearrange("(k p) o -> p k o", p=128),
                )
                ident_sb = consts.tile([128, 128], F16)
                nc.sync.dma_start(out=ident_sb, in_=ident_d.ap())
                c0_sb = consts.tile([128, J], F16)
                nc.sync.dma_start(out=c0_sb, in_=c0_d.ap())
                sig0_sb = consts.tile([1, 40], F16)
                nc.sync.dma_start(out=sig0_sb, in_=sig0_d.ap())
                ones10_sb = consts.tile([1, J], F16)
                nc.sync.dma_start(out=ones10_sb, in_=ones10_d.ap())
                ones40_sb = consts.tile([1, 40], F16)
                nc.sync.dma_start(out=ones40_sb, in_=ones40_d.ap())
                onesrow_sb = consts.tile([1, 128], F16)
                nc.sync.dma_start(out=onesrow_sb, in_=onesrow_d.ap())
                ones1_sb = consts.tile([128, 1], F16)
                nc.sync.dma_start(out=ones1_sb, in_=ones1_d.ap())
                dmask_sb = consts.tile([128, D], F16)
                nc.sync.dma_start(out=dmask_sb, in_=dmask_d.ap())
                bmask_sb = consts.tile([128, 30], F16)
                nc.sync.dma_start(out=bmask_sb, in_=bmask_d.ap())
                bmask0_sb = consts.tile([128, 30], F16)
                nc.sync.dma_start(out=bmask0_sb, in_=bmask0_d.ap())
                # conv bias, f32 staging -> f16 row + 3-col layouts
                wbf_sb = consts.tile([1, O], F32)
                nc.sync.dma_start(out=wbf_sb, in_=wb_d.ap())
                wbc_f = consts.tile([128, 3], F32)
                for ob, olo in enumerate(OLOS):
                    nc.sync.dma_start(
                        out=wbc_f[0:128, ob:ob + 1],
                        in_=wb_d.ap()[0:1, olo:olo + 128],
                    )
                wbrow_sb = consts.tile([1, O], F16)
                nc.vector.tensor_copy(wbrow_sb, wbf_sb)
                wbcol_sb = consts.tile([128, 3], F16)
                nc.vector.tensor_copy(wbcol_sb, wbc_f)
                CK.update(wt=wt_sb, ident=ident_sb, c0=c0_sb, sig0=sig0_sb,
                          ones10=ones10_sb,
                          ones40=ones40_sb, onesrow=onesrow_sb, ones1=ones1_sb,
                          dmask=dmask_sb, bmask=bmask_sb, bmask0=bmask0_sb,
                          wbrow=wbrow_sb, wbcol=wbcol_sb)

            # ---------------- conv + transposes, all batches ----------------
            predT = [None] * B_PER_CORE
            predB = [None] * B_PER_CORE

            def conv_batch(b):
                x_sb = xp.tile([128, 2 * I], F32R, tag="x")
                nc.sync.dma_start(
                    out=x_sb.rearrange("p (k i) -> p k i", i=I),
                    in_=x_d.ap()[b].rearrange("(k p) i -> p k i", p=128),
                )
                if b == 0:
                    load_consts()
                wt_sb = CK["wt"]
                pT = ptp.tile([128, N_IT * O], F16, tag="pT")
                for pair in range(4):
                    cv = pscv.tile([128, 1024], F32, tag="cv")
                    for h in range(2):
                        it = pair * 2 + h
                        for k in range(2):
                            nc.tensor.matmul(
                                cv[:, h * 512:h * 512 + O],
                                x_sb[:, k * I + it * 128:k * I + it * 128 + 128],
                                wt_sb[:, k * O:(k + 1) * O],
                                start=(k == 0), stop=(k == 1),
                            )
                    src = ap_of(cv, 0, [[512, 2], [1, O]])
                    dst = pT[:, pair * 640:pair * 640 + 640]
                    nc.scalar.copy(dst, src)
                predT[b] = pT

                pB = pbp.tile([128, 3 * I], F16, tag="pB")
                for ob, olo in enumerate(OLOS):
                    tp = pstp.tile([128, I], F16, tag="tp")
                    for it in range(N_IT):
                        nc.tensor.transpose(
                            tp[:, it * 128:(it + 1) * 128],
                            pT[:, it * O + olo:it * O + olo + 128],
                            CK["ident"],
                        )
                    dst = pB[:, ob * I:(ob + 1) * I]
                    nc.vector.tensor_copy(dst, tp)
                predB[b] = pB

            for b in range(B_PER_CORE):
                conv_batch(b)
            if dump:
                nc.sync.dma_start(out=DT["predT0"].ap(), in_=predT[0])
                nc.sync.dma_start(out=DT["predB0"].ap(), in_=predB[0])

            # ---------------- routing (wave-synchronous, interleaved) -------
            b_sb = [None] * N_WAVES
            c_sb = [None] * N_WAVES

            def rout_iter(w, t):
                last = t == ROUTE_NUM - 1
                bmask = CK["bmask0"] if t == 0 else CK["bmask"]
                rt = psrt.tile([128, 512], F32, tag="rt")
                # region map (cols), lifetimes sequenced by subtile deps:
                #   s 0:120 -> ns 120:160, ns+1 160:200 (row 0)
                #   -> bc 120:160 (full 128 rows) -> bv 160:200 (row 0)
                #   -> pd 0:320 (t<2) or v 0:128 (t=2)
                if t > 0:
                    e_sb = st.tile([128, WAVE * N_IT * J], F16, tag="e")
                    nc.scalar.activation(e_sb, b_sb[w], AF.Exp)
                    den = st.tile([128, WAVE * N_IT], F16, tag="den")
                    nc.vector.reduce_sum(
                        den, e_sb.rearrange("p (g j) -> p g j", j=J), axis=AX.X)
                    rden = st.tile([128, WAVE * N_IT], F16, tag="rden")
                    nc.vector.reciprocal(rden, den)
                    c = st.tile([128, WAVE * N_IT * J], F16, tag="c")
                    nc.gpsimd.tensor_tensor(
                        c.rearrange("p (g j) -> p g j", j=J),
                        e_sb.rearrange("p (g j) -> p g j", j=J),
                        rden.broadcast_to([128, WAVE * N_IT, J]),
                        OP.mult,
                    )
                    c_sb[w] = c
                    if dump and w == 0:
                        nc.sync.dma_start(out=DT[f"c{t}"].ap(), in_=c)

                # sigma[b,j] = sum_i c[i,j] (softmax normalizes over j, so this
                # is ~I/J, not 1): partition-sum via ones matmul, then reduce
                # the 8 i-tiles on DVE. Scales the bias rank-1 update below.
                if t == 0:
                    sig = CK["sig0"]
                else:
                    nc.tensor.matmul(rt[0:1, 160:480], CK["ones1"], c_sb[w],
                                     start=True, stop=True)
                    sig = st.tile([1, 40], F16, tag="sig")
                    nc.vector.reduce_sum(
                        sig.rearrange("p (b j) -> p b j", j=J),
                        ap_of(rt, 160, [[N_IT * J, WAVE], [1, J], [J, N_IT]])[0:1],
                        axis=AX.X,
                    )

                # s^T: out [o-block, 10] per (b4, ob); bias as K=1 start matmul
                for b4 in range(WAVE):
                    gb = w * WAVE + b4
                    for ob, olo in enumerate(OLOS):
                        out_w = rt[:, (b4 * 3 + ob) * J:(b4 * 3 + ob + 1) * J]
                        nc.tensor.matmul(
                            out_w, CK["wbrow"][0:1, olo:olo + 128],
                            sig[0:1, b4 * J:(b4 + 1) * J],
                            start=True, stop=False,
                        )
                        for it in range(N_IT):
                            rhs = (CK["c0"] if t == 0 else
                                   c_sb[w][:, (b4 * N_IT + it) * J:(b4 * N_IT + it + 1) * J])
                            nc.tensor.matmul(
                                out_w,
                                predT[gb][:, it * O + olo:it * O + olo + 128],
                                rhs,
                                start=False, stop=(it == N_IT - 1),
                            )

                # mask -> s_m (f16), square, norms -> rt[0:2,120:160]
                s_m = st.tile([128, WAVE * 30], F16, tag="s_m")
                nc.vector.tensor_tensor(
                    s_m.rearrange("p (b m) -> p b m", m=30),
                    ap_of(rt, 0, [[30, WAVE], [1, 30]]),
                    ap_of(bmask, 0, [[0, WAVE], [1, 30]]),
                    OP.mult,
                )
                sq = st.tile([128, WAVE * 30], F16, tag="sq")
                nc.gpsimd.tensor_tensor(sq, s_m, s_m, OP.mult)
                # ns into [120:160]; ns+1 into [160:200] (starts from a rank-1
                # matmul of ones so the +1 rides the accumulation group)
                for ob in range(3):
                    nc.tensor.matmul(
                        rt[0:1, 120:160], CK["ones1"],
                        ap_of(sq, ob * J, [[30, WAVE], [1, J]]),
                        start=(ob == 0), stop=(ob == 2),
                    )
                nc.tensor.matmul(rt[0:1, 160:200], CK["onesrow"][0:1, 0:1],
                                 CK["ones40"], start=True, stop=False)
                for ob in range(3):
                    nc.tensor.matmul(
                        rt[0:1, 160:200], CK["ones1"],
                        ap_of(sq, ob * J, [[30, WAVE], [1, J]]),
                        start=False, stop=(ob == 2),
                    )
                # coeff = exp(0.5 ln ns - ln(1+ns)) : [1,40] f16
                lns2 = st.tile([1, 80], F32, tag="lns2")
                nc.scalar.activation(lns2, rt[0:1, 120:200], AF.Ln)
                tv = st.tile([1, 40], F32, tag="tv")
                nc.vector.scalar_tensor_tensor(
                    tv, lns2[0:1, 0:40], 0.5, lns2[0:1, 40:80],
                    OP.mult, OP.subtract)
                coeff = st.tile([1, 40], F16, tag="coeff")
                nc.scalar.activation(coeff, tv, AF.Exp)
                if dump and w == 0:
                    nc.sync.dma_start(out=DT[f"sm{t}"].ap(), in_=s_m)
                    ns_c = st.tile([1, 80], F32, tag="ns_c")
                    nc.vector.tensor_copy(ns_c, rt[0:1, 120:200])
                    nc.sync.dma_start(out=DT[f"ns{t}"].ap(), in_=ns_c)
                    nc.sync.dma_start(out=DT[f"co{t}"].ap(), in_=coeff)
                # broadcast coeff down partitions: rank-1 matmul -> rt[:,120:160]
                nc.tensor.matmul(rt[:, 120:160], CK["onesrow"], coeff,
                                 start=True, stop=True)
                V = st.tile([128, WAVE * 30], F16, tag="V")
                nc.vector.tensor_tensor(
                    V.rearrange("p (b k j) -> p b k j", k=3, j=J),
                    s_m.rearrange("p (b k j) -> p b k j", k=3, j=J),
                    ap_of(rt, 120, [[J, WAVE], [0, 3], [1, J]]),
                    OP.mult,
                )
                if dump and w == 0:
                    nc.sync.dma_start(out=DT[f"V{t}"].ap(), in_=V)

                if not last:
                    # bv[b,j] = sum_o bias[o] V[o,(b,j)]  (bias term of delta)
                    for ob in range(3):
                        nc.tensor.matmul(
                            rt[0:1, 160:200], CK["wbcol"][:, ob:ob + 1],
                            ap_of(V, ob * J, [[30, WAVE], [1, J]]),
                            start=(ob == 0), stop=(ob == 2),
                        )
                    bv = st.tile([1, 40], F16, tag="bv")
                    nc.vector.tensor_copy(bv, rt[0:1, 160:200])
                    if dump and w == 0:
                        nc.sync.dma_start(out=DT[f"bv{t}"].ap(), in_=bv)
                    # delta: out [i-part, 10] per (b4, it2); overlays dead s/bc
                    for b4 in range(WAVE):
                        gb = w * WAVE + b4
                        for it2 in range(N_IT):
                            dw = rt[:, (b4 * N_IT + it2) * J:
                                    (b4 * N_IT + it2 + 1) * J]
                            nc.tensor.matmul(
                                dw, CK["onesrow"], bv[0:1, b4 * J:(b4 + 1) * J],
                                start=True, stop=False,
                            )
                            for ob in range(3):
                                nc.tensor.matmul(
                                    dw,
                                    predB[gb][:, ob * I + it2 * 128:ob * I + it2 * 128 + 128],
                                    V[:, (b4 * 3 + ob) * J:(b4 * 3 + ob + 1) * J],
                                    start=False, stop=(ob == 2),
                                )
                    if t == 0:
                        bb = st.tile([128, WAVE * N_IT * J], F32, tag="b")
                        nc.vector.tensor_copy(bb, rt[:, 0:320])
                        b_sb[w] = bb
                    else:
                        nc.vector.tensor_tensor(
                            b_sb[w], rt[:, 0:320], b_sb[w], OP.add)
                    if dump and w == 0:
                        nc.sync.dma_start(out=DT[f"b{t}"].ap(), in_=b_sb[w])
                else:
                    # v[j,d] via dmask: out [10,32] per b4
                    for b4 in range(WAVE):
                        for ob in range(3):
                            nc.tensor.matmul(
                                rt[0:J, b4 * D:(b4 + 1) * D],
                                V[:, (b4 * 3 + ob) * J:(b4 * 3 + ob + 1) * J],
                                CK["dmask"],
                                start=(ob == 0), stop=(ob == 2),
                            )
                    vout = st.tile([J, WAVE * D], F32, tag="vout")
                    nc.vector.tensor_copy(vout, rt[0:J, 0:WAVE * D])
                    oap = out_d.ap()
                    dst = bass.AP(
                        tensor=oap.tensor, offset=w * WAVE * J * D,
                        ap=[[D, J], [J * D, WAVE], [1, D]],
                    )
                    nc.sync.dma_start(out=dst, in_=vout)

            for t in range(ROUTE_NUM):
                for w in range(N_WAVES):
                    rout_iter(w, t)

    nc.compile()
    return nc


_NC_CACHE = None
LAST_RESULT = None


def kernel(x: np.ndarray, W: np.ndarray, W_b: np.ndarray) -> np.ndarray:
    global _NC_CACHE
    if _NC_CACHE is None:
        _NC_CACHE = build_kernel()
    nc = _NC_CACHE

    x = np.ascontiguousarray(x.reshape(BS, C_IN, I), dtype=np.float32)
    wt = np.ascontiguousarray(W.T, dtype=np.float32)
    wb = np.ascontiguousarray(W_b.reshape(1, O), dtype=np.float32)

    in_maps = [
        {
            "x": np.ascontiguousarray(x[c * B_PER_CORE:(c + 1) * B_PER_CORE]),
            "wt": wt,
            "wb": wb,
        }
        for c in range(N_CORES)
    ]
    import os
    trace = bool(int(os.environ.get("KERNEL_TRACE", "0")))
    res = run_bass_kernel_spmd(
        nc, in_maps, core_ids=list(range(N_CORES)), trace=trace
    )
    if trace:
        global LAST_RESULT
        LAST_RESULT = res
    out = np.concatenate([res.results[c]["v"] for c in range(N_CORES)], axis=0)
    return out.astype(np.float32)


if __name__ == "__main__":
    rng = np.random.default_rng(0)
    x = rng.standard_normal((BS, C_IN, 32, 32), dtype=np.float32)
    W = (rng.standard_normal((O, C_IN)) * 0.02).astype(np.float32)
    W_b = (rng.standard_normal((O,)) * 0.02).astype(np.float32)
    v = kernel(x=x, W=W, W_b=W_b)
    print(v.shape, v.dtype, float(np.abs(v).max()))


# revision 33
# speedup vs baseline: 1.6135x; 1.0027x over previous
"""Trainium2 Bass kernel for CapLayer2 (1x1-conv capsule layer with dynamic routing).

Sharding: data-parallel over batch - 8 batches per core on 8 NeuronCores.

Design notes (driven by the instruction cost model):
  - Matmul cost = out-free-size x pe_cycle; weight (stationary) loads are free.
    So every routing contraction puts the big tensor in the STATIONARY operand
    and streams a 10-column moving operand: s^T = predT^T c (out [o,10]),
    delta = predB^T V (out [i,10]), v = V^T dmask (out [10,32]).
  - The conv computes only predT [i-part, o]; the [o-part, i] layout (predB)
    comes from PE transposes of the f16 predT (1.0 cyc/row, psum out f16),
    whose eviction then runs at the DVE 2x half-cycle rate.
  - All on-chip tensors are float16 (f16 matmuls are full rate; fp32r at
    free-size<256 would be 4x slower; f16 keeps 10 mantissa bits).
  - The conv bias is folded into the routing matmuls as rank-1 updates
    (K=1 matmuls), so all PSUM evictions are pure copies and can be spread
    across DVE/ACT/GPSIMD.
  - o-blocks are [0:128), [128:256), [192:320): all 128 partitions wide.
    The V mask zeroes rows of block1 that alias block2, so the block-2
    overlap contributes zero to delta/v and no PSUM row is ever read stale.
  - squash coeff: ns -> [2,40] (duplicated rows via ones2 lhsT), one ACT Ln
    with per-partition bias (0,1) gives ln(ns) and ln(1+ns), one DVE
    scalar_tensor_tensor forms 0.5*ln(ns)-ln(1+ns), one ACT Exp gives
    coeff = sqrt(ns)/(1+ns); a rank-1 matmul broadcasts it to 128 partitions.
"""

import numpy as np
import ml_dtypes

import concourse.bacc as bacc
import concourse.bass as bass
import concourse.hw_specs as hw_specs

# Keep every activation in one function table so no table switches are emitted.
_ONE_TABLE = "natural_log_exp_and_others"
_orig_get_tables = hw_specs.get_activation_tables


def _pinned_tables(arch):
    tabs = _orig_get_tables(arch)
    return {k: (v if k == _ONE_TABLE else set()) for k, v in tabs.items()}


bacc.get_activation_tables = _pinned_tables
import concourse.tile as tile
from concourse import mybir
from concourse.bass_utils import run_bass_kernel_spmd

F32 = mybir.dt.float32
F32R = mybir.dt.float32r
F16 = mybir.dt.float16
AF = mybir.ActivationFunctionType
OP = mybir.AluOpType
AX = mybir.AxisListType

N_CORES = 8
BS = 64
C_IN = 256
J = 10
D = 32
O = J * D          # 320
I = 1024
ROUTE_NUM = 3
B_PER_CORE = BS // N_CORES   # 8
WAVE = 4
N_WAVES = B_PER_CORE // WAVE
N_IT = I // 128    # 8

# o-blocks for the [o-part] layouts: all 128 wide; block1 rows >=64 (o 192..255)
# are masked out of V so block2 [192:320) supplies them instead.
OLOS = [0, 128, 192]
F16T = np.float16


def ap_of(t, offset_cols, dims):
    """Custom free-dim AP over tile t: dims = [[stride, count], ...]."""
    return bass.AP(tensor=t.tensor, offset=t.offset + offset_cols,
                   ap=[list(t.ap[0])] + [list(d) for d in dims])


def build_kernel(dump=False):
    nc = bacc.Bacc("TRN2", target_bir_lowering=False, debug=False, num_devices=1)

    x_d = nc.dram_tensor("x", [B_PER_CORE, C_IN, I], F32R, kind="ExternalInput")
    wt_d = nc.dram_tensor("wt", [C_IN, O], F32R, kind="ExternalInput")   # W.T
    wb_d = nc.dram_tensor("wb", [1, O], F32, kind="ExternalInput")
    out_d = nc.dram_tensor("v", [B_PER_CORE, J, D], F32, kind="ExternalOutput")
    DT = {}
    if dump:
        DT["predT0"] = nc.dram_tensor("predT0", [128, N_IT * O], F16, kind="ExternalOutput")
        DT["predB0"] = nc.dram_tensor("predB0", [128, 3 * I], F16, kind="ExternalOutput")
        for t in range(ROUTE_NUM):
            DT[f"sm{t}"] = nc.dram_tensor(f"sm{t}", [128, 120], F16, kind="ExternalOutput")
            DT[f"ns{t}"] = nc.dram_tensor(f"ns{t}", [1, 80], F32, kind="ExternalOutput")
            DT[f"co{t}"] = nc.dram_tensor(f"co{t}", [1, 40], F16, kind="ExternalOutput")
            DT[f"V{t}"] = nc.dram_tensor(f"V{t}", [128, 120], F16, kind="ExternalOutput")
        for t in range(2):
            DT[f"b{t}"] = nc.dram_tensor(f"b{t}", [128, 320], F32, kind="ExternalOutput")
            DT[f"bv{t}"] = nc.dram_tensor(f"bv{t}", [1, 40], F16, kind="ExternalOutput")
        for t in (1, 2):
            DT[f"c{t}"] = nc.dram_tensor(f"c{t}", [128, 320], F16, kind="ExternalOutput")

    # ---- inline constants (f16) ----
    ident_np = np.eye(128, dtype=F16T)
    c0v = np.float16(0.1)                      # f16(0.1) = 0.0999755859375
    c0_np = np.full((128, J), c0v, dtype=F16T)
    corr = 0.1 / float(c0v)                    # iter-0 softmax correction
    # sigma0 = sum_i c0[i,j] = 1024*f16(0.1); the bmask0 correction then maps
    # it to the exact 102.4 the f32 reference uses.
    sig0_np = np.full((1, 40), np.float32(I) * c0v, dtype=F16T)
    ones10_np = np.ones((1, J), dtype=F16T)
    ones40_np = np.ones((1, 40), dtype=F16T)
    onesrow_np = np.ones((1, 128), dtype=F16T)
    ones1_np = np.ones((128, 1), dtype=F16T)
    dmask_np = np.zeros((128, D), dtype=F16T)
    for p in range(128):
        dmask_np[p, p % D] = 1.0
    bmask_np = np.zeros((128, 30), dtype=np.float32)
    for ob, olo in enumerate(OLOS):
        for p in range(128):
            o = olo + p
            if ob == 1 and o >= 192:
                continue                       # block2 owns o in [192,320)
            bmask_np[p, ob * J + o // D] = 1.0
    ident_d = nc.inline_tensor(ident_np, name="ident")
    c0_d = nc.inline_tensor(c0_np, name="c0")
    sig0_d = nc.inline_tensor(sig0_np, name="sig0")
    ones10_d = nc.inline_tensor(ones10_np, name="ones10")
    ones40_d = nc.inline_tensor(ones40_np, name="ones40")
    onesrow_d = nc.inline_tensor(onesrow_np, name="onesrow")
    ones1_d = nc.inline_tensor(ones1_np, name="ones1")
    dmask_d = nc.inline_tensor(dmask_np, name="dmask")
    bmask_d = nc.inline_tensor(bmask_np.astype(F16T), name="bmask")
    bmask0_d = nc.inline_tensor((bmask_np * corr).astype(F16T), name="bmask0")

    with tile.TileContext(nc) as tc:
        with nc.allow_low_precision(reason="f16 intermediates, 2e-2 tolerance"), \
             tc.tile_pool(name="consts", bufs=1) as consts, \
             tc.tile_pool(name="xp", bufs=3) as xp, \
             tc.tile_pool(name="ptp", bufs=B_PER_CORE) as ptp, \
             tc.tile_pool(name="pbp", bufs=B_PER_CORE) as pbp, \
             tc.tile_pool(name="st", bufs=2) as st, \
             tc.tile_pool(name="pscv", bufs=2, space="PSUM") as pscv, \
             tc.tile_pool(name="pstp", bufs=2, space="PSUM") as pstp, \
             tc.tile_pool(name="psrt", bufs=2, space="PSUM") as psrt:

            # ---------------- constants ----------------
            CK = {}

            def load_consts():
                wt_sb = consts.tile([128, 2 * O], F32R)
                nc.sync.dma_start(
                    out=wt_sb.rearrange("p (k o) -> p k o", o=O),
                    in_=wt_d.ap().rearrange("(k p) o -> p k o", p=128),
                )
                ident_sb = consts.tile([128, 128], F16)
                nc.sync.dma_start(out=ident_sb, in_=ident_d.ap())
                c0_sb = consts.tile([128, J], F16)
                nc.sync.dma_start(out=c0_sb, in_=c0_d.ap())
                sig0_sb = consts.tile([1, 40], F16)
                nc.sync.dma_start(out=sig0_sb, in_=sig0_d.ap())
                ones10_sb = consts.tile([1, J], F16)
                nc.sync.dma_start(out=ones10_sb, in_=ones10_d.ap())
                ones40_sb = consts.tile([1, 40], F16)
                nc.sync.dma_start(out=ones40_sb, in_=ones40_d.ap())
                onesrow_sb = consts.tile([1, 128], F16)
                nc.sync.dma_start(out=onesrow_sb, in_=onesrow_d.ap())
                ones1_sb = consts.tile([128, 1], F16)
                nc.sync.dma_start(out=ones1_sb, in_=ones1_d.ap())
                dmask_sb = consts.tile([128, D], F16)
                nc.sync.dma_start(out=dmask_sb, in_=dmask_d.ap())
                bmask_sb = consts.tile([128, 30], F16)
                nc.sync.dma_start(out=bmask_sb, in_=bmask_d.ap())
                bmask0_sb = consts.tile([128, 30], F16)
                nc.sync.dma_start(out=bmask0_sb, in_=bmask0_d.ap())
                # conv bias, f32 staging -> f16 row + 3-col layouts
                wbf_sb = consts.tile([1, O], F32)
                nc.sync.dma_start(out=wbf_sb, in_=wb_d.ap())
                wbc_f = consts.tile([128, 3], F32)
                for ob, olo in enumerate(OLOS):
                    nc.sync.dma_start(
                        out=wbc_f[0:128, ob:ob + 1],
                        in_=wb_d.ap()[0:1, olo:olo + 128],
                    )
                wbrow_sb = consts.tile([1, O], F16)
                nc.vector.tensor_copy(wbrow_sb, wbf_sb)
                wbcol_sb = consts.tile([128, 3], F16)
                nc.vector.tensor_copy(wbcol_sb, wbc_f)
                CK.update(wt=wt_sb, ident=ident_sb, c0=c0_sb, sig0=sig0_sb,
                          ones10=ones10_sb,
                          ones40=ones40_sb, onesrow=onesrow_sb, ones1=ones1_sb,
                          dmask=dmask_sb, bmask=bmask_sb, bmask0=bmask0_sb,
                          wbrow=wbrow_sb, wbcol=wbcol_sb)

            # ---------------- conv + transposes, all batches ----------------
            predT = [None] * B_PER_CORE
            predB = [None] * B_PER_CORE

            def conv_mms(b):
                x_sb = xp.tile([128, 2 * I], F32R, tag="x")
                nc.sync.dma_start(
                    out=x_sb.rearrange("p (k i) -> p k i", i=I),
                    in_=x_d.ap()[b].rearrange("(k p) i -> p k i", p=128),
                )
                if b == 0:
                    load_consts()
                wt_sb = CK["wt"]
                pT = ptp.tile([128, N_IT * O], F16, tag="pT")
                for pair in range(4):
                    cv = pscv.tile([128, 1024], F32, tag="cv")
                    for h in range(2):
                        it = pair * 2 + h
                        for k in range(2):
                            nc.tensor.matmul(
                                cv[:, h * 512:h * 512 + O],
                                x_sb[:, k * I + it * 128:k * I + it * 128 + 128],
                                wt_sb[:, k * O:(k + 1) * O],
                                start=(k == 0), stop=(k == 1),
                            )
                    src = ap_of(cv, 0, [[512, 2], [1, O]])
                    dst = pT[:, pair * 640:pair * 640 + 640]
                    nc.scalar.copy(dst, src)
                predT[b] = pT

            def conv_tp(b):
                pT = predT[b]
                pB = pbp.tile([128, 3 * I], F16, tag="pB")
                for ob, olo in enumerate(OLOS):
                    tp = pstp.tile([128, I], F16, tag="tp")
                    for it in range(N_IT):
                        nc.tensor.transpose(
                            tp[:, it * 128:(it + 1) * 128],
                            pT[:, it * O + olo:it * O + olo + 128],
                            CK["ident"],
                        )
                    dst = pB[:, ob * I:(ob + 1) * I]
                    nc.vector.tensor_copy(dst, tp)
                predB[b] = pB

            # ---------------- routing (wave-synchronous, interleaved) -------
            b_sb = [None] * N_WAVES
            c_sb = [None] * N_WAVES

            def rout_iter(w, t):
                last = t == ROUTE_NUM - 1
                bmask = CK["bmask0"] if t == 0 else CK["bmask"]
                rt = psrt.tile([128, 512], F32, tag="rt")
                # region map (cols), lifetimes sequenced by subtile deps:
                #   s 0:120 -> ns 120:160, ns+1 160:200 (row 0)
                #   -> bc 120:160 (full 128 rows) -> bv 160:200 (row 0)
                #   -> pd 0:320 (t<2) or v 0:128 (t=2)
                if t > 0:
                    e_sb = st.tile([128, WAVE * N_IT * J], F16, tag="e")
                    nc.scalar.activation(e_sb, b_sb[w], AF.Exp)
                    den = st.tile([128, WAVE * N_IT], F16, tag="den")
                    nc.vector.reduce_sum(
                        den, e_sb.rearrange("p (g j) -> p g j", j=J), axis=AX.X)
                    rden = st.tile([128, WAVE * N_IT], F16, tag="rden")
                    nc.vector.reciprocal(rden, den)
                    c = st.tile([128, WAVE * N_IT * J], F16, tag="c")
                    nc.gpsimd.tensor_tensor(
                        c.rearrange("p (g j) -> p g j", j=J),
                        e_sb.rearrange("p (g j) -> p g j", j=J),
                        rden.broadcast_to([128, WAVE * N_IT, J]),
                        OP.mult,
                    )
                    c_sb[w] = c
                    if dump and w == 0:
                        nc.sync.dma_start(out=DT[f"c{t}"].ap(), in_=c)

                # sigma[b,j] = sum_i c[i,j] (softmax normalizes over j, so this
                # is ~I/J, not 1): partition-sum via ones matmul, then reduce
                # the 8 i-tiles on DVE. Scales the bias rank-1 update below.
                if t == 0:
                    sig = CK["sig0"]
                else:
                    nc.tensor.matmul(rt[0:1, 160:480], CK["ones1"], c_sb[w],
                                     start=True, stop=True)
                    sig = st.tile([1, 40], F16, tag="sig")
                    nc.vector.reduce_sum(
                        sig.rearrange("p (b j) -> p b j", j=J),
                        ap_of(rt, 160, [[N_IT * J, WAVE], [1, J], [J, N_IT]])[0:1],
                        axis=AX.X,
                    )

                # s^T: out [o-block, 10] per (b4, ob); bias as K=1 start matmul
                for b4 in range(WAVE):
                    gb = w * WAVE + b4
                    for ob, olo in enumerate(OLOS):
                        out_w = rt[:, (b4 * 3 + ob) * J:(b4 * 3 + ob + 1) * J]
                        nc.tensor.matmul(
                            out_w, CK["wbrow"][0:1, olo:olo + 128],
                            sig[0:1, b4 * J:(b4 + 1) * J],
                            start=True, stop=False,
                        )
                        for it in range(N_IT):
                            rhs = (CK["c0"] if t == 0 else
                                   c_sb[w][:, (b4 * N_IT + it) * J:(b4 * N_IT + it + 1) * J])
                            nc.tensor.matmul(
                                out_w,
                                predT[gb][:, it * O + olo:it * O + olo + 128],
                                rhs,
                                start=False, stop=(it == N_IT - 1),
                            )

                # mask -> s_m (f16), square, norms -> rt[0:2,120:160]
                s_m = st.tile([128, WAVE * 30], F16, tag="s_m")
                nc.vector.tensor_tensor(
                    s_m.rearrange("p (b m) -> p b m", m=30),
                    ap_of(rt, 0, [[30, WAVE], [1, 30]]),
                    ap_of(bmask, 0, [[0, WAVE], [1, 30]]),
                    OP.mult,
                )
                sq = st.tile([128, WAVE * 30], F16, tag="sq")
                nc.gpsimd.tensor_tensor(sq, s_m, s_m, OP.mult)
                # ns into [120:160]; ns+1 into [160:200] (starts from a rank-1
                # matmul of ones so the +1 rides the accumulation group)
                for ob in range(3):
                    nc.tensor.matmul(
                        rt[0:1, 120:160], CK["ones1"],
                        ap_of(sq, ob * J, [[30, WAVE], [1, J]]),
                        start=(ob == 0), stop=(ob == 2),
                    )
                nc.tensor.matmul(rt[0:1, 160:200], CK["onesrow"][0:1, 0:1],
                                 CK["ones40"], start=True, stop=False)
                for ob in range(3):
                    nc.tensor.matmul(
                        rt[0:1, 160:200], CK["ones1"],
                        ap_of(sq, ob * J, [[30, WAVE], [1, J]]),
                        start=False, stop=(ob == 2),
                    )
                # coeff = exp(0.5 ln ns - ln(1+ns)) : [1,40] f16
                lns2 = st.tile([1, 80], F32, tag="lns2")
                nc.scalar.activation(lns2, rt[0:1, 120:200], AF.Ln)
                tv = st.tile([1, 40], F32, tag="tv")
                nc.vector.scalar_tensor_tensor(
                    tv, lns2[0:1, 0:40], 0.5, lns2[0:1, 40:80],
                    OP.mult, OP.subtract)
                coeff = st.tile([1, 40], F16, tag="coeff")
                nc.scalar.activation(coeff, tv, AF.Exp)
                if dump and w == 0:
                    nc.sync.dma_start(out=DT[f"sm{t}"].ap(), in_=s_m)
                    ns_c = st.tile([1, 80], F32, tag="ns_c")
                    nc.vector.tensor_copy(ns_c, rt[0:1, 120:200])
                    nc.sync.dma_start(out=DT[f"ns{t}"].ap(), in_=ns_c)
                    nc.sync.dma_start(out=DT[f"co{t}"].ap(), in_=coeff)
                # broadcast coeff down partitions: rank-1 matmul -> rt[:,120:160]
                nc.tensor.matmul(rt[:, 120:160], CK["onesrow"], coeff,
                                 start=True, stop=True)
                V = st.tile([128, WAVE * 30], F16, tag="V")
                nc.vector.tensor_tensor(
                    V.rearrange("p (b k j) -> p b k j", k=3, j=J),
                    s_m.rearrange("p (b k j) -> p b k j", k=3, j=J),
                    ap_of(rt, 120, [[J, WAVE], [0, 3], [1, J]]),
                    OP.mult,
                )
                if dump and w == 0:
                    nc.sync.dma_start(out=DT[f"V{t}"].ap(), in_=V)

                if not last:
                    # bv[b,j] = sum_o bias[o] V[o,(b,j)]  (bias term of delta)
                    for ob in range(3):
                        nc.tensor.matmul(
                            rt[0:1, 160:200], CK["wbcol"][:, ob:ob + 1],
                            ap_of(V, ob * J, [[30, WAVE], [1, J]]),
                            start=(ob == 0), stop=(ob == 2),
                        )
                    bv = st.tile([1, 40], F16, tag="bv")
                    nc.vector.tensor_copy(bv, rt[0:1, 160:200])
                    if dump and w == 0:
                        nc.sync.dma_start(out=DT[f"bv{t}"].ap(), in_=bv)
                    # delta: out [i-part, 10] per (b4, it2); overlays dead s/bc
                    for b4 in range(WAVE):
                        gb = w * WAVE + b4
                        for it2 in range(N_IT):
                            dw = rt[:, (b4 * N_IT + it2) * J:
                                    (b4 * N_IT + it2 + 1) * J]
                            nc.tensor.matmul(
                                dw, CK["onesrow"], bv[0:1, b4 * J:(b4 + 1) * J],
                                start=True, stop=False,
                            )
                            for ob in range(3):
                                nc.tensor.matmul(
                                    dw,
                                    predB[gb][:, ob * I + it2 * 128:ob * I + it2 * 128 + 128],
                                    V[:, (b4 * 3 + ob) * J:(b4 * 3 + ob + 1) * J],
                                    start=False, stop=(ob == 2),
                                )
                    if t == 0:
                        bb = st.tile([128, WAVE * N_IT * J], F32, tag="b")
                        nc.vector.tensor_copy(bb, rt[:, 0:320])
                        b_sb[w] = bb
                    else:
                        nc.vector.tensor_tensor(
                            b_sb[w], rt[:, 0:320], b_sb[w], OP.add)
                    if dump and w == 0:
                        nc.sync.dma_start(out=DT[f"b{t}"].ap(), in_=b_sb[w])
                else:
                    # v[j,d] via dmask: out [10,32] per b4
                    for b4 in range(WAVE):
                        for ob in range(3):
                            nc.tensor.matmul(
                                rt[0:J, b4 * D:(b4 + 1) * D],
                                V[:, (b4 * 3 + ob) * J:(b4 * 3 + ob + 1) * J],
                                CK["dmask"],
                                start=(ob == 0), stop=(ob == 2),
                            )
                    vout = st.tile([J, WAVE * D], F32, tag="vout")
                    nc.vector.tensor_copy(vout, rt[0:J, 0:WAVE * D])
                    oap = out_d.ap()
                    dst = bass.AP(
                        tensor=oap.tensor, offset=w * WAVE * J * D,
                        ap=[[D, J], [J * D, WAVE], [1, D]],
                    )
                    nc.sync.dma_start(out=dst, in_=vout)

            # Software-pipelined emission: tp(b) lands after conv_mms(b+1) so
            # PE never waits on the ACT evictions of the same batch; routing
            # iterations are interleaved into the conv stream so routing ops
            # don't queue behind all 8 batches' evictions on in-order engines.
            conv_mms(0)
            conv_mms(1)
            conv_tp(0)
            conv_mms(2)
            conv_tp(1)
            conv_mms(3)
            conv_tp(2)
            conv_mms(4)
            conv_tp(3)
            rout_iter(0, 0)
            conv_mms(5)
            conv_tp(4)
            rout_iter(0, 1)
            conv_mms(6)
            conv_tp(5)
            conv_mms(7)
            conv_tp(6)
            conv_tp(7)
            rout_iter(1, 0)
            rout_iter(0, 2)
            rout_iter(1, 1)
            rout_iter(1, 2)
            if dump:
                nc.sync.dma_start(out=DT["predT0"].ap(), in_=predT[0])
                nc.sync.dma_start(out=DT["predB0"].ap(), in_=predB[0])

    nc.compile()
    return nc


_NC_CACHE = None
LAST_RESULT = None


def kernel(x: np.ndarray, W: np.ndarray, W_b: np.ndarray) -> np.ndarray:
    global _NC_CACHE
    if _NC_CACHE is None:
        _NC_CACHE = build_kernel()
    nc = _NC_CACHE

    x = np.ascontiguousarray(x.reshape(BS, C_IN, I), dtype=np.float32)
    wt = np.ascontiguousarray(W.T, dtype=np.float32)
    wb = np.ascontiguousarray(W_b.reshape(1, O), dtype=np.float32)

    in_maps = [
        {
            "x": np.ascontiguousarray(x[c * B_PER_CORE:(c + 1) * B_PER_CORE]),
            "wt": wt,
            "wb": wb,
        }
        for c in range(N_CORES)
    ]
    import os
    trace = bool(int(os.environ.get("KERNEL_TRACE", "0")))
    res = run_bass_kernel_spmd(
        nc, in_maps, core_ids=list(range(N_CORES)), trace=trace
    )
    if trace:
        global LAST_RESULT
        LAST_RESULT = res
    out = np.concatenate([res.results[c]["v"] for c in range(N_CORES)], axis=0)
    return out.astype(np.float32)


if __name__ == "__main__":
    rng = np.random.default_rng(0)
    x = rng.standard_normal((BS, C_IN, 32, 32), dtype=np.float32)
    W = (rng.standard_normal((O, C_IN)) * 0.02).astype(np.float32)
    W_b = (rng.standard_normal((O,)) * 0.02).astype(np.float32)
    v = kernel(x=x, W=W, W_b=W_b)
    print(v.shape, v.dtype, float(np.abs(v).max()))


# revision 42
# speedup vs baseline: 1.7259x; 1.0696x over previous
"""Trainium2 Bass kernel for CapLayer2 (1x1-conv capsule layer with dynamic routing).

Sharding: data-parallel over batch - 8 batches per core on 8 NeuronCores.

Design notes (driven by the instruction cost model):
  - Matmul cost = out-free-size x pe_cycle; weight (stationary) loads are free.
    So every routing contraction puts the big tensor in the STATIONARY operand
    and streams a 10-column moving operand: s^T = predT^T c (out [o,10]),
    delta = predB^T V (out [i,10]), v = V^T dmask (out [10,32]).
  - The conv computes only predT [i-part, o]; the [o-part, i] layout (predB)
    comes from PE transposes of the f16 predT (1.0 cyc/row, psum out f16),
    whose eviction then runs at the DVE 2x half-cycle rate.
  - All on-chip tensors are float16 (f16 matmuls are full rate; fp32r at
    free-size<256 would be 4x slower; f16 keeps 10 mantissa bits).
  - The conv bias is folded into the routing matmuls as rank-1 updates
    (K=1 matmuls), so all PSUM evictions are pure copies and can be spread
    across DVE/ACT/GPSIMD.
  - o-blocks are [0:128), [128:256), [192:320): all 128 partitions wide.
    The V mask zeroes rows of block1 that alias block2, so the block-2
    overlap contributes zero to delta/v and no PSUM row is ever read stale.
  - squash coeff: ns -> [2,40] (duplicated rows via ones2 lhsT), one ACT Ln
    with per-partition bias (0,1) gives ln(ns) and ln(1+ns), one DVE
    scalar_tensor_tensor forms 0.5*ln(ns)-ln(1+ns), one ACT Exp gives
    coeff = sqrt(ns)/(1+ns); a rank-1 matmul broadcasts it to 128 partitions.
"""

import numpy as np
import ml_dtypes

import concourse.bacc as bacc
import concourse.bass as bass
import concourse.hw_specs as hw_specs

# Keep every activation in one function table so no table switches are emitted.
_ONE_TABLE = "natural_log_exp_and_others"
_orig_get_tables = hw_specs.get_activation_tables


def _pinned_tables(arch):
    tabs = _orig_get_tables(arch)
    return {k: (v if k == _ONE_TABLE else set()) for k, v in tabs.items()}


bacc.get_activation_tables = _pinned_tables
import concourse.tile as tile
from concourse import mybir
from concourse.bass_utils import run_bass_kernel_spmd

F32 = mybir.dt.float32
F32R = mybir.dt.float32r
F16 = mybir.dt.float16
AF = mybir.ActivationFunctionType
OP = mybir.AluOpType
AX = mybir.AxisListType

N_CORES = 8
BS = 64
C_IN = 256
J = 10
D = 32
O = J * D          # 320
I = 1024
ROUTE_NUM = 3
B_PER_CORE = BS // N_CORES   # 8
WAVE = 4
N_WAVES = B_PER_CORE // WAVE
N_IT = I // 128    # 8

# o-blocks for the [o-part] layouts: all 128 wide; block1 rows >=64 (o 192..255)
# are masked out of V so block2 [192:320) supplies them instead.
OLOS = [0, 128, 192]
F16T = np.float16


def ap_of(t, offset_cols, dims):
    """Custom free-dim AP over tile t: dims = [[stride, count], ...]."""
    return bass.AP(tensor=t.tensor, offset=t.offset + offset_cols,
                   ap=[list(t.ap[0])] + [list(d) for d in dims])


def build_kernel(dump=False):
    nc = bacc.Bacc("TRN2", target_bir_lowering=False, debug=False, num_devices=1)

    x_d = nc.dram_tensor("x", [B_PER_CORE, C_IN, I], F32R, kind="ExternalInput")
    wt_d = nc.dram_tensor("wt", [C_IN, O], F32R, kind="ExternalInput")   # W.T
    wb_d = nc.dram_tensor("wb", [1, O], F32, kind="ExternalInput")
    out_d = nc.dram_tensor("v", [B_PER_CORE, J, D], F32, kind="ExternalOutput")
    DT = {}
    if dump:
        DT["predT0"] = nc.dram_tensor("predT0", [128, N_IT * O], F16, kind="ExternalOutput")
        DT["predB0"] = nc.dram_tensor("predB0", [128, 3 * I], F16, kind="ExternalOutput")
        for t in range(ROUTE_NUM):
            DT[f"sm{t}"] = nc.dram_tensor(f"sm{t}", [128, 120], F16, kind="ExternalOutput")
            DT[f"ns{t}"] = nc.dram_tensor(f"ns{t}", [1, 80], F32, kind="ExternalOutput")
            DT[f"co{t}"] = nc.dram_tensor(f"co{t}", [1, 40], F16, kind="ExternalOutput")
            DT[f"V{t}"] = nc.dram_tensor(f"V{t}", [128, 120], F16, kind="ExternalOutput")
        for t in range(2):
            DT[f"b{t}"] = nc.dram_tensor(f"b{t}", [128, 320], F32, kind="ExternalOutput")
            DT[f"bv{t}"] = nc.dram_tensor(f"bv{t}", [1, 40], F16, kind="ExternalOutput")
        for t in (1, 2):
            DT[f"c{t}"] = nc.dram_tensor(f"c{t}", [128, 320], F16, kind="ExternalOutput")

    # ---- inline constants (f16) ----
    ident_np = np.eye(128, dtype=F16T)
    c0v = np.float16(0.1)                      # f16(0.1) = 0.0999755859375
    c0_np = np.full((128, J), c0v, dtype=F16T)
    corr = 0.1 / float(c0v)                    # iter-0 softmax correction
    # sigma0 = sum_i c0[i,j] = 1024*f16(0.1); the bmask0 correction then maps
    # it to the exact 102.4 the f32 reference uses.
    sig0_np = np.full((1, 40), np.float32(I) * c0v, dtype=F16T)
    ones10_np = np.ones((1, J), dtype=F16T)
    ones40_np = np.ones((1, 40), dtype=F16T)
    onesrow_np = np.ones((1, 128), dtype=F16T)
    ones1_np = np.ones((128, 1), dtype=F16T)
    dmask_np = np.zeros((128, D), dtype=F16T)
    for p in range(128):
        dmask_np[p, p % D] = 1.0
    bmask_np = np.zeros((128, 30), dtype=np.float32)
    for ob, olo in enumerate(OLOS):
        for p in range(128):
            o = olo + p
            if ob == 1 and o >= 192:
                continue                       # block2 owns o in [192,320)
            bmask_np[p, ob * J + o // D] = 1.0
    # Pack every static f16 constant into one [128, N] tensor -> ONE DMA
    # (tiny separate DMAs each hold the serialized DGE path ~650ns).
    packs = {}
    cols = 0
    pieces = []
    for nm, arr in [
        ("ident", ident_np), ("c0", c0_np), ("sig0", sig0_np),
        ("ones10", ones10_np), ("ones40", ones40_np),
        ("onesrow", onesrow_np), ("ones1", ones1_np), ("dmask", dmask_np),
        ("bmask", bmask_np.astype(F16T)),
        ("bmask0", (bmask_np * corr).astype(F16T)),
    ]:
        p, n = arr.shape
        full = np.zeros((128, n), dtype=F16T)
        full[:p] = arr
        packs[nm] = (cols, p, n)
        pieces.append(full)
        cols += n
    pack_d = nc.inline_tensor(np.concatenate(pieces, axis=1), name="cpack")

    with tile.TileContext(nc) as tc:
        with nc.allow_low_precision(reason="f16 intermediates, 2e-2 tolerance"), \
             tc.tile_pool(name="consts", bufs=1) as consts, \
             tc.tile_pool(name="xp", bufs=B_PER_CORE) as xp, \
             tc.tile_pool(name="ptp", bufs=B_PER_CORE) as ptp, \
             tc.tile_pool(name="pbp", bufs=B_PER_CORE) as pbp, \
             tc.tile_pool(name="st", bufs=2) as st, \
             tc.tile_pool(name="pscv", bufs=2, space="PSUM") as pscv, \
             tc.tile_pool(name="pstp", bufs=2, space="PSUM") as pstp, \
             tc.tile_pool(name="psrt", bufs=2, space="PSUM") as psrt:

            # ---------------- constants ----------------
            CK = {}

            def load_consts():
                wt_sb = consts.tile([128, 2 * O], F32R)
                nc.sync.dma_start(
                    out=wt_sb.rearrange("p (k o) -> p k o", o=O),
                    in_=wt_d.ap().rearrange("(k p) o -> p k o", p=128),
                )
                pack_sb = consts.tile([128, cols], F16)
                nc.sync.dma_start(out=pack_sb, in_=pack_d.ap())
                CK["wt"] = wt_sb
                for nm, (c0c, p, n) in packs.items():
                    CK[nm] = pack_sb[0:p, c0c:c0c + n]

            def load_wb():
                # conv bias (runtime input), f32 staging -> f16 row + col forms
                wbf_sb = consts.tile([1, O], F32)
                nc.sync.dma_start(out=wbf_sb, in_=wb_d.ap())
                wbc_f = consts.tile([128, 3], F32)
                for ob, olo in enumerate(OLOS):
                    nc.sync.dma_start(
                        out=wbc_f[0:128, ob:ob + 1],
                        in_=wb_d.ap()[0:1, olo:olo + 128],
                    )
                wbrow_sb = consts.tile([1, O], F16)
                nc.vector.tensor_copy(wbrow_sb, wbf_sb)
                wbcol_sb = consts.tile([128, 3], F16)
                nc.vector.tensor_copy(wbcol_sb, wbc_f)
                CK.update(wbrow=wbrow_sb, wbcol=wbcol_sb)

            # ---------------- conv + transposes, all batches ----------------
            predT = [None] * B_PER_CORE
            predB = [None] * B_PER_CORE

            x_tiles = [None] * B_PER_CORE

            def load_x(b):
                x_sb = xp.tile([128, 2 * I], F32R, tag="x")
                nc.sync.dma_start(
                    out=x_sb.rearrange("p (k i) -> p k i", i=I),
                    in_=x_d.ap()[b].rearrange("(k p) i -> p k i", p=128),
                )
                x_tiles[b] = x_sb

            def conv_mms(b):
                x_sb = x_tiles[b]
                wt_sb = CK["wt"]
                pT = ptp.tile([128, N_IT * O], F16, tag="pT")
                for pair in range(4):
                    cv = pscv.tile([128, 1024], F32, tag="cv")
                    for h in range(2):
                        it = pair * 2 + h
                        for k in range(2):
                            nc.tensor.matmul(
                                cv[:, h * 512:h * 512 + O],
                                x_sb[:, k * I + it * 128:k * I + it * 128 + 128],
                                wt_sb[:, k * O:(k + 1) * O],
                                start=(k == 0), stop=(k == 1),
                            )
                    src = ap_of(cv, 0, [[512, 2], [1, O]])
                    dst = pT[:, pair * 640:pair * 640 + 640]
                    nc.scalar.copy(dst, src)
                predT[b] = pT

            def conv_tp(b):
                pT = predT[b]
                pB = pbp.tile([128, 3 * I], F16, tag="pB")
                for ob, olo in enumerate(OLOS):
                    tp = pstp.tile([128, I], F16, tag="tp")
                    for it in range(N_IT):
                        nc.tensor.transpose(
                            tp[:, it * 128:(it + 1) * 128],
                            pT[:, it * O + olo:it * O + olo + 128],
                            CK["ident"],
                        )
                    dst = pB[:, ob * I:(ob + 1) * I]
                    nc.vector.tensor_copy(dst, tp)
                predB[b] = pB

            # ---------------- routing (wave-synchronous, interleaved) -------
            b_sb = [None] * N_WAVES
            c_sb = [None] * N_WAVES

            def rout_iter(w, t):
                last = t == ROUTE_NUM - 1
                bmask = CK["bmask0"] if t == 0 else CK["bmask"]
                rt = psrt.tile([128, 512], F32, tag="rt")
                # region map (cols), lifetimes sequenced by subtile deps:
                #   s 0:120 -> ns 120:160, ns+1 160:200 (row 0)
                #   -> bc 120:160 (full 128 rows) -> bv 160:200 (row 0)
                #   -> pd 0:320 (t<2) or v 0:128 (t=2)
                if t > 0:
                    e_sb = st.tile([128, WAVE * N_IT * J], F16, tag="e")
                    nc.scalar.activation(e_sb, b_sb[w], AF.Exp)
                    den = st.tile([128, WAVE * N_IT], F16, tag="den")
                    nc.vector.reduce_sum(
                        den, e_sb.rearrange("p (g j) -> p g j", j=J), axis=AX.X)
                    rden = st.tile([128, WAVE * N_IT], F16, tag="rden")
                    nc.vector.reciprocal(rden, den)
                    c = st.tile([128, WAVE * N_IT * J], F16, tag="c")
                    nc.vector.tensor_tensor(
                        c.rearrange("p (g j) -> p g j", j=J),
                        e_sb.rearrange("p (g j) -> p g j", j=J),
                        rden.broadcast_to([128, WAVE * N_IT, J]),
                        OP.mult,
                    )
                    c_sb[w] = c
                    if dump and w == 0:
                        nc.sync.dma_start(out=DT[f"c{t}"].ap(), in_=c)

                # sigma[b,j] = sum_i c[i,j] (softmax normalizes over j, so this
                # is ~I/J, not 1): partition-sum via ones matmul, then reduce
                # the 8 i-tiles on DVE. Scales the bias rank-1 update below.
                if t == 0:
                    sig = CK["sig0"]
                else:
                    nc.tensor.matmul(rt[0:1, 160:480], CK["ones1"], c_sb[w],
                                     start=True, stop=True)
                    sig = st.tile([1, 40], F16, tag="sig")
                    nc.vector.reduce_sum(
                        sig.rearrange("p (b j) -> p b j", j=J),
                        ap_of(rt, 160, [[N_IT * J, WAVE], [1, J], [J, N_IT]])[0:1],
                        axis=AX.X,
                    )

                # s^T: out [o-block, 10] per (b4, ob); bias (scaled by sigma)
                # rides the group as the K=1 STOP matmul so the c-matmuls
                # never wait on sigma.
                for b4 in range(WAVE):
                    gb = w * WAVE + b4
                    for ob, olo in enumerate(OLOS):
                        out_w = rt[:, (b4 * 3 + ob) * J:(b4 * 3 + ob + 1) * J]
                        for it in range(N_IT):
                            rhs = (CK["c0"] if t == 0 else
                                   c_sb[w][:, (b4 * N_IT + it) * J:(b4 * N_IT + it + 1) * J])
                            nc.tensor.matmul(
                                out_w,
                                predT[gb][:, it * O + olo:it * O + olo + 128],
                                rhs,
                                start=(it == 0), stop=False,
                            )
                        nc.tensor.matmul(
                            out_w, CK["wbrow"][0:1, olo:olo + 128],
                            sig[0:1, b4 * J:(b4 + 1) * J],
                            start=False, stop=True,
                        )

                # mask -> s_m (f16), square, norms -> rt[0:2,120:160]
                s_m = st.tile([128, WAVE * 30], F16, tag="s_m")
                nc.vector.tensor_tensor(
                    s_m.rearrange("p (b m) -> p b m", m=30),
                    ap_of(rt, 0, [[30, WAVE], [1, 30]]),
                    ap_of(bmask, 0, [[0, WAVE], [1, 30]]),
                    OP.mult,
                )
                sq = st.tile([128, WAVE * 30], F16, tag="sq")
                nc.vector.tensor_tensor(sq, s_m, s_m, OP.mult)
                # ns into [120:160]; ns+1 into [160:200] (starts from a rank-1
                # matmul of ones so the +1 rides the accumulation group)
                for ob in range(3):
                    nc.tensor.matmul(
                        rt[0:1, 120:160], CK["ones1"],
                        ap_of(sq, ob * J, [[30, WAVE], [1, J]]),
                        start=(ob == 0), stop=(ob == 2),
                    )
                nc.tensor.matmul(rt[0:1, 160:200], CK["onesrow"][0:1, 0:1],
                                 CK["ones40"], start=True, stop=False)
                for ob in range(3):
                    nc.tensor.matmul(
                        rt[0:1, 160:200], CK["ones1"],
                        ap_of(sq, ob * J, [[30, WAVE], [1, J]]),
                        start=False, stop=(ob == 2),
                    )
                # coeff = exp(0.5 ln ns - ln(1+ns)) : [1,40] f16
                lns2 = st.tile([1, 80], F32, tag="lns2")
                nc.scalar.activation(lns2, rt[0:1, 120:200], AF.Ln)
                tv = st.tile([1, 40], F32, tag="tv")
                nc.vector.scalar_tensor_tensor(
                    tv, lns2[0:1, 0:40], 0.5, lns2[0:1, 40:80],
                    OP.mult, OP.subtract)
                coeff = st.tile([1, 40], F16, tag="coeff")
                nc.scalar.activation(coeff, tv, AF.Exp)
                if dump and w == 0:
                    nc.sync.dma_start(out=DT[f"sm{t}"].ap(), in_=s_m)
                    ns_c = st.tile([1, 80], F32, tag="ns_c")
                    nc.vector.tensor_copy(ns_c, rt[0:1, 120:200])
                    nc.sync.dma_start(out=DT[f"ns{t}"].ap(), in_=ns_c)
                    nc.sync.dma_start(out=DT[f"co{t}"].ap(), in_=coeff)
                # broadcast coeff down partitions: rank-1 matmul -> rt[:,120:160]
                nc.tensor.matmul(rt[:, 120:160], CK["onesrow"], coeff,
                                 start=True, stop=True)
                V = st.tile([128, WAVE * 30], F16, tag="V")
                nc.vector.tensor_tensor(
                    V.rearrange("p (b k j) -> p b k j", k=3, j=J),
                    s_m.rearrange("p (b k j) -> p b k j", k=3, j=J),
                    ap_of(rt, 120, [[J, WAVE], [0, 3], [1, J]]),
                    OP.mult,
                )
                if dump and w == 0:
                    nc.sync.dma_start(out=DT[f"V{t}"].ap(), in_=V)

                if not last:
                    # bv[b,j] = sum_o bias[o] V[o,(b,j)]  (bias term of delta)
                    for ob in range(3):
                        nc.tensor.matmul(
                            rt[0:1, 160:200], CK["wbcol"][:, ob:ob + 1],
                            ap_of(V, ob * J, [[30, WAVE], [1, J]]),
                            start=(ob == 0), stop=(ob == 2),
                        )
                    bv = st.tile([1, 40], F16, tag="bv")
                    nc.vector.tensor_copy(bv, rt[0:1, 160:200])
                    if dump and w == 0:
                        nc.sync.dma_start(out=DT[f"bv{t}"].ap(), in_=bv)
                    # delta: out [i-part, 10] per (b4, it2); overlays dead s/bc;
                    # the bias rank-1 update is the STOP matmul (waits on bv,
                    # the raw matmuls only wait on V).
                    for b4 in range(WAVE):
                        gb = w * WAVE + b4
                        for it2 in range(N_IT):
                            dw = rt[:, (b4 * N_IT + it2) * J:
                                    (b4 * N_IT + it2 + 1) * J]
                            for ob in range(3):
                                nc.tensor.matmul(
                                    dw,
                                    predB[gb][:, ob * I + it2 * 128:ob * I + it2 * 128 + 128],
                                    V[:, (b4 * 3 + ob) * J:(b4 * 3 + ob + 1) * J],
                                    start=(ob == 0), stop=False,
                                )
                            nc.tensor.matmul(
                                dw, CK["onesrow"], bv[0:1, b4 * J:(b4 + 1) * J],
                                start=False, stop=True,
                            )
                    if t == 0:
                        bb = st.tile([128, WAVE * N_IT * J], F32, tag="b")
                        nc.vector.tensor_copy(bb, rt[:, 0:320])
                        b_sb[w] = bb
                    else:
                        nc.vector.tensor_tensor(
                            b_sb[w], rt[:, 0:320], b_sb[w], OP.add)
                    if dump and w == 0:
                        nc.sync.dma_start(out=DT[f"b{t}"].ap(), in_=b_sb[w])
                else:
                    # v[j,d] via dmask: out [10,32] per b4
                    for b4 in range(WAVE):
                        for ob in range(3):
                            nc.tensor.matmul(
                                rt[0:J, b4 * D:(b4 + 1) * D],
                                V[:, (b4 * 3 + ob) * J:(b4 * 3 + ob + 1) * J],
                                CK["dmask"],
                                start=(ob == 0), stop=(ob == 2),
                            )
                    vout = st.tile([J, WAVE * D], F32, tag="vout")
                    nc.vector.tensor_copy(vout, rt[0:J, 0:WAVE * D])
                    oap = out_d.ap()
                    dst = bass.AP(
                        tensor=oap.tensor, offset=w * WAVE * J * D,
                        ap=[[D, J], [J * D, WAVE], [1, D]],
                    )
                    nc.sync.dma_start(out=dst, in_=vout)

            # DMA issue order: x0, consts, x1-x4, wb, x5-x7 (the serialized DMA
            # device paces everything; small transfers go early but never in
            # front of an x tile that gates compute).
            load_x(0)
            load_consts()
            for b in range(1, 5):
                load_x(b)
            load_wb()
            for b in range(5, B_PER_CORE):
                load_x(b)
            # Software-pipelined emission: tp(b) lands after conv_mms(b+1) so
            # PE never waits on the ACT evictions of the same batch; routing
            # iterations are interleaved into the conv stream so routing ops
            # don't queue behind all 8 batches' evictions on in-order engines.
            conv_mms(0)
            conv_mms(1)
            conv_tp(0)
            conv_mms(2)
            conv_tp(1)
            conv_mms(3)
            conv_tp(2)
            conv_mms(4)
            conv_tp(3)
            rout_iter(0, 0)
            conv_mms(5)
            conv_tp(4)
            rout_iter(0, 1)
            conv_mms(6)
            conv_tp(5)
            conv_mms(7)
            conv_tp(6)
            conv_tp(7)
            rout_iter(1, 0)
            rout_iter(0, 2)
            rout_iter(1, 1)
            rout_iter(1, 2)
            if dump:
                nc.sync.dma_start(out=DT["predT0"].ap(), in_=predT[0])
                nc.sync.dma_start(out=DT["predB0"].ap(), in_=predB[0])

    nc.compile()
    return nc


_NC_CACHE = None
LAST_RESULT = None


def kernel(x: np.ndarray, W: np.ndarray, W_b: np.ndarray) -> np.ndarray:
    global _NC_CACHE
    if _NC_CACHE is None:
        _NC_CACHE = build_kernel()
    nc = _NC_CACHE

    x = np.ascontiguousarray(x.reshape(BS, C_IN, I), dtype=np.float32)
    wt = np.ascontiguousarray(W.T, dtype=np.float32)
    wb = np.ascontiguousarray(W_b.reshape(1, O), dtype=np.float32)

    in_maps = [
        {
            "x": np.ascontiguousarray(x[c * B_PER_CORE:(c + 1) * B_PER_CORE]),
            "wt": wt,
            "wb": wb,
        }
        for c in range(N_CORES)
    ]
    import os
    trace = bool(int(os.environ.get("KERNEL_TRACE", "0")))
    res = run_bass_kernel_spmd(
        nc, in_maps, core_ids=list(range(N_CORES)), trace=trace
    )
    if trace:
        global LAST_RESULT
        LAST_RESULT = res
    out = np.concatenate([res.results[c]["v"] for c in range(N_CORES)], axis=0)
    return out.astype(np.float32)


if __name__ == "__main__":
    rng = np.random.default_rng(0)
    x = rng.standard_normal((BS, C_IN, 32, 32), dtype=np.float32)
    W = (rng.standard_normal((O, C_IN)) * 0.02).astype(np.float32)
    W_b = (rng.standard_normal((O,)) * 0.02).astype(np.float32)
    v = kernel(x=x, W=W, W_b=W_b)
    print(v.shape, v.dtype, float(np.abs(v).max()))
